# revision 11
# baseline (speedup 1.0000x reference)
"""Trainium2 Bass kernel for nn_Conv2dKan (KAN 3x3 conv, Hermite basis 8 + silu residual).

Full-input contract: kernel(x, w_b, w_s, c) -> [16, 128, 32, 32] fp32.

Math:
  out[b,o,l] = sum_{i,k,a} (w_s*c)[i,o,k,a] * H_a(xw[b,i,k,l])
             + sum_{i,k}   w_b[i,o,k]      * silu(xw[b,i,k,l])
  where xw = 3x3 unfold of x with zero padding 1.

Kernel strategy:
  - Re-parametrize Hermite basis into monomials x^m (m=0..7) by folding the
    (exact, integer) Hermite coefficient matrix into the weights host-side.
  - The m=0 (constant) feature contributes a position-independent per-channel
    bias (valid at padding too, since x^m(0)=0 for m>=1), added at the end.
  - On chip per core (2 images): zero-padded x in SBUF [128p=(img,cin), 34*34],
    features x^1..x^7 (DVE muls) + silu (ACT), cast to bf16; then 3x3 conv as
    accumulated matmuls: contraction (cin=64) per (feature m, tap k, image),
    K=64 matmuls at row groups 0/64 run pairwise-concurrently on the PE.
  - Data parallel over batch: 16 images / 8 cores.
"""

import numpy as np
import ml_dtypes

import concourse.bacc as bacc
import concourse.bass as bass
import concourse.mybir as mybir
import concourse.tile as tile
from concourse.bass_utils import run_bass_kernel_spmd

F32 = mybir.dt.float32
BF16 = mybir.dt.bfloat16

B, CIN, H, W = 16, 64, 32, 32
COUT = 128
K2 = 9          # 3x3 taps
BASIS = 8       # Hermite orders 0..7
NFEAT = 8       # on-chip features: x^1..x^7, silu
NCORES = 8
IMGS_PER_CORE = B // NCORES  # 2
HP, WP = H + 2, W + 2        # padded 34x34
LP = HP * WP                 # 1156
L = H * W                    # 1024
NHALF = 512                  # psum free dim (half the image)

_CACHE = {}


def _hermite_coeff_matrix():
    """C[a, m] = coefficient of x^m in physicists' Hermite H_a, a,m in 0..7."""
    C = np.zeros((BASIS, BASIS), dtype=np.float64)
    C[0, 0] = 1.0
    C[1, 1] = 2.0
    for n in range(1, BASIS - 1):
        # H_{n+1} = 2 x H_n - 2 n H_{n-1}
        C[n + 1, 1:] += 2.0 * C[n, :-1]
        C[n + 1, :] -= 2.0 * n * C[n - 1, :]
    return C


def _build_program():
    """Build + compile the per-core Bass program (cached per process)."""
    if "nc" in _CACHE:
        return _CACHE["nc"]

    nc = bacc.Bacc("TRN2", target_bir_lowering=False, debug=False,
                   num_devices=NCORES)

    x_in = nc.dram_tensor("x_in", [IMGS_PER_CORE, CIN, H, W], F32,
                          kind="ExternalInput").ap()
    # weight layout: [p, (j*K2 + k)*COUT + o]; p<64 -> feature f0(j) chan p,
    # p>=64 -> feature f1(j) chan p-64
    w_in = nc.dram_tensor("w_in", [128, NPAIR * K2 * COUT], BF16,
                          kind="ExternalInput").ap()
    b_in = nc.dram_tensor("b_in", [COUT, 1], F32, kind="ExternalInput").ap()
    y_out = nc.dram_tensor("y_out", [IMGS_PER_CORE, COUT, L], F32,
                           kind="ExternalOutput").ap()

    with tile.TileContext(nc) as tc:
        _kernel_body(nc, tc, x_in, w_in, b_in, y_out)

    nc.compile()
    _CACHE["nc"] = nc
    return nc


NPAIR = 4  # feature pairs per image: (x1,x2) (x3,x4) (x5,x6) (x7,silu)


def _kernel_body(nc, tc, x_in, w_in, b_in, y_out):
    """Feature pairs packed on partitions -> all matmuls are K=128 (FWL-fast
    weight loads, full PE row utilization). Per image r, pair tile j holds
    feature f0(j) on partitions 0..63 and f1(j) on 64..127 (bf16)."""
    with (
        tc.tile_pool(name="wpool", bufs=1) as wpool,
        tc.tile_pool(name="fpool", bufs=1) as fpool,
        tc.tile_pool(name="iopool", bufs=2) as iopool,
        tc.tile_pool(name="psum", bufs=4, space="PSUM") as ppool,
    ):
        # --- weight chunk j=0 first (first matmul group needs it)
        wt = [None] * NPAIR
        w_0 = wpool.tile([128, K2 * COUT], BF16, name="w_0")
        nc.sync.dma_start(w_0, w_in[:, 0:K2 * COUT])
        wt[0] = w_0

        # --- per image: duplicated zero-padded input on both partition halves
        xpd_, d_, sig_ = [], [], []
        B = [[None] * NPAIR for _ in range(IMGS_PER_CORE)]
        for r in range(IMGS_PER_CORE):
            xpd = fpool.tile([128, LP], F32, name=f"xpd{r}")
            nc.gpsimd.memset(xpd, 0.0)
            xp3 = xpd.rearrange("p (h w) -> p h w", w=WP)
            for half in range(2):
                nc.sync.dma_start(
                    xp3[half * CIN:(half + 1) * CIN, 1:H + 1, 1:W + 1], x_in[r])
            xpd_.append(xpd)

        for r in range(IMGS_PER_CORE):
            d = fpool.tile([128, LP], F32, name=f"d{r}")        # [x^2; x^2]
            nc.scalar.activation(d, xpd_[r], mybir.ActivationFunctionType.Square)
            d_.append(d)
        # pair 0 for both images first, so j=0 matmuls can start ASAP
        for r in range(IMGS_PER_CORE):
            b1 = fpool.tile([128, LP], BF16, name=f"b1_{r}")    # [x; x^2]
            nc.vector.tensor_copy(b1[:CIN, :], xpd_[r][:CIN, :])
            nc.vector.tensor_copy(b1[CIN:, :], d_[r][CIN:, :])
            B[r][0] = b1

        # remaining weight chunks
        for j in range(1, NPAIR):
            w_j = wpool.tile([128, K2 * COUT], BF16, name=f"w_{j}")
            nc.sync.dma_start(w_j, w_in[:, j * K2 * COUT:(j + 1) * K2 * COUT])
            wt[j] = w_j
        bias = wpool.tile([COUT, 1], F32, name="bias")
        nc.sync.dma_start(bias, b_in)

        for r in range(IMGS_PER_CORE):
            sig = fpool.tile([128, LP], F32, name=f"sig{r}")
            nc.scalar.activation(sig[CIN:, :], xpd_[r][CIN:, :],
                                 mybir.ActivationFunctionType.Sigmoid)
            sig_.append(sig)

        p2_, p3_ = [], []
        for r in range(IMGS_PER_CORE):
            xpd, d = xpd_[r], d_[r]
            p2 = fpool.tile([128, LP], F32, name=f"p2_{r}")     # [x^3; x^4]
            nc.vector.tensor_mul(p2[:CIN, :], xpd[:CIN, :], d[:CIN, :])
            nc.vector.tensor_mul(p2[CIN:, :], d[CIN:, :], d[CIN:, :])
            b2 = fpool.tile([128, LP], BF16, name=f"b2_{r}")
            nc.vector.tensor_copy(b2, p2)
            B[r][1] = b2
            p2_.append(p2)
        for r in range(IMGS_PER_CORE):
            p3 = fpool.tile([128, LP], F32, name=f"p3_{r}")     # [x^5; x^6]
            nc.vector.tensor_mul(p3, p2_[r], d_[r])
            b3 = fpool.tile([128, LP], BF16, name=f"b3_{r}")
            nc.vector.tensor_copy(b3, p3)
            B[r][2] = b3
            p3_.append(p3)
        for r in range(IMGS_PER_CORE):
            b4 = fpool.tile([128, LP], BF16, name=f"b4_{r}")    # [x^7; silu]
            nc.vector.tensor_mul(b4[:CIN, :], p3_[r][:CIN, :], d_[r][:CIN, :])
            nc.vector.tensor_mul(b4[CIN:, :], sig_[r][CIN:, :], xpd_[r][CIN:, :])
            B[r][3] = b4

        # --- conv as accumulated K=128 matmuls
        n_acc = NPAIR * K2  # matmuls per psum tile
        for nh in range(2):  # output row halves (16 rows x 32 cols = 512)
            psums = [ppool.tile([COUT, NHALF], F32, name=f"ps{nh}_{r}", tag="ps")
                     for r in range(IMGS_PER_CORE)]
            for r in range(IMGS_PER_CORE):
                for j in range(NPAIR):
                    for k in range(K2):
                        kh, kw = divmod(k, 3)
                        cnt = j * K2 + k
                        lhsT = wt[j][:, k * COUT:(k + 1) * COUT]
                        g3 = B[r][j].rearrange("p (h w) -> p h w", w=WP)
                        rhs = g3[:, nh * 16 + kh: nh * 16 + kh + 16, kw: kw + W]
                        nc.tensor.matmul(psums[r], lhsT, rhs,
                                         start=(cnt == 0),
                                         stop=(cnt == n_acc - 1))
            for r in range(IMGS_PER_CORE):
                o_sb = iopool.tile([COUT, NHALF], F32, name=f"osb{nh}_{r}",
                                   tag="osb")
                nc.vector.tensor_scalar(o_sb, psums[r], bias, None,
                                        op0=mybir.AluOpType.add)
                nc.sync.dma_start(y_out[r, :, nh * NHALF:(nh + 1) * NHALF],
                                  o_sb)


def _prepare_host_inputs(x, w_b, w_s, c):
    """Fold Hermite->monomial transform into weights; build per-core inputs."""
    x = np.asarray(x, dtype=np.float32)
    w_b64 = np.asarray(w_b, dtype=np.float64)[..., 0]          # [i,o,k]
    w_s64 = np.asarray(w_s, dtype=np.float64)[..., 0]          # [i,o,k]
    c64 = np.asarray(c, dtype=np.float64)[:, :, :, 0, :]       # [i,o,k,a]

    cw = w_s64[..., None] * c64                                # [i,o,k,a]
    C = _hermite_coeff_matrix()                                # [a,m]
    w_mono = np.einsum("ioka,am->iokm", cw, C)                 # [i,o,k,m]

    bias = w_mono[..., 0].sum(axis=(0, 2)).astype(np.float32)  # [o]

    # w_host[p, (j*K2 + k)*COUT + o]: pair j = (f0, f1) on partition halves
    # j=0..2 -> (x^{2j+1}, x^{2j+2}); j=3 -> (x^7, silu)
    w_host = np.zeros((128, NPAIR * K2 * COUT), dtype=np.float64)
    for j in range(NPAIR):
        f0 = w_mono[:, :, :, 2 * j + 1]                        # [i,o,k]
        f1 = w_mono[:, :, :, 2 * j + 2] if j < 3 else w_b64
        blk0 = np.transpose(f0, (0, 2, 1)).reshape(CIN, K2 * COUT)
        blk1 = np.transpose(f1, (0, 2, 1)).reshape(CIN, K2 * COUT)
        w_host[:CIN, j * K2 * COUT:(j + 1) * K2 * COUT] = blk0
        w_host[CIN:, j * K2 * COUT:(j + 1) * K2 * COUT] = blk1
    w_host = w_host.astype(ml_dtypes.bfloat16)

    in_maps = []
    for core in range(NCORES):
        xs = x[core * IMGS_PER_CORE:(core + 1) * IMGS_PER_CORE]
        in_maps.append({
            "x_in": np.ascontiguousarray(xs),
            "w_in": w_host,
            "b_in": bias.reshape(COUT, 1),
        })
    return in_maps, w_host.astype(np.float64), bias


def _spot_reference(x, w_host64, bias, b_idx, n_out=16):
    """Numpy mini-reference for one image, first n_out channels (kernel math)."""
    xp = np.zeros((CIN, HP, WP), dtype=np.float64)
    xp[:, 1:H + 1, 1:W + 1] = x[b_idx].astype(np.float64)
    feats = []
    for j in range(NPAIR):
        f0 = xp ** (2 * j + 1)
        f1 = xp ** (2 * j + 2) if j < 3 else xp / (1.0 + np.exp(-xp))
        feats.append(np.concatenate([f0, f1], axis=0))   # [128, HP, WP]
    out = np.tile(bias[:n_out, None].astype(np.float64), (1, L))  # [n_out, L]
    for j in range(NPAIR):
        for k in range(K2):
            kh, kw = divmod(k, 3)
            win = feats[j][:, kh:kh + H, kw:kw + W].reshape(128, L)
            wk = w_host64[:, (j * K2 + k) * COUT:(j * K2 + k) * COUT + n_out]
            out += wk.T @ win
    return out  # [n_out, L] float64


def kernel(x, w_b, w_s, c):
    nc = _build_program()
    in_maps, w_host64, bias = _prepare_host_inputs(x, w_b, w_s, c)
    x = np.asarray(x, dtype=np.float32)

    for _attempt in range(3):
        res = run_bass_kernel_spmd(nc, in_maps, core_ids=list(range(NCORES)))
        out = np.concatenate(
            [res.results[core]["y_out"].reshape(IMGS_PER_CORE, COUT, H, W)
             for core in range(NCORES)], axis=0).astype(np.float32)
        # guard against transient device garbage: spot-check 1 image per core
        ok = np.isfinite(out).all()
        if ok:
            for core in range(NCORES):
                b_idx = core * IMGS_PER_CORE
                ref = _spot_reference(x, w_host64, bias, b_idx)
                got = out[b_idx, :16].reshape(16, L).astype(np.float64)
                err = np.linalg.norm(got - ref) / (np.linalg.norm(ref) + 1e-30)
                if not np.isfinite(err) or err > 3e-2:
                    ok = False
                    break
        if ok:
            return out
    raise RuntimeError("kernel: device output failed spot-check after 3 attempts")


# revision 15
# speedup vs baseline: 1.1404x; 1.1404x over previous
"""Trainium2 Bass kernel for nn_Conv2dKan (KAN 3x3 conv, Hermite basis 8 + silu residual).

Full-input contract: kernel(x, w_b, w_s, c) -> [16, 128, 32, 32] fp32.

Math:
  out[b,o,l] = sum_{i,k,a} (w_s*c)[i,o,k,a] * H_a(xw[b,i,k,l])
             + sum_{i,k}   w_b[i,o,k]      * silu(xw[b,i,k,l])
  where xw = 3x3 unfold of x with zero padding 1.

Kernel strategy:
  - Re-parametrize Hermite basis into monomials x^m (m=0..7) by folding the
    (exact, integer) Hermite coefficient matrix into the weights host-side.
  - The m=0 (constant) feature contributes a position-independent per-channel
    bias (valid at padding too, since x^m(0)=0 for m>=1), added at the end.
  - On chip per core (2 images): per image, a zero-padded duplicated input
    tile [128p = x twice, 34*34]; feature PAIRS packed on partition halves
    ((x1,x2) (x3,x4) (x5,x6) (x7,silu)) built with DVE muls + ACT
    square/sigmoid, cast bf16. The 3x3 conv is then accumulated K=128
    matmuls (bf16 in, fp32 PSUM): 4 pairs x 9 taps x 2 images x 2 spatial
    halves (N=512) = 144 matmuls/core, rhs windows read via strided APs
    (no unfold materialization; Hermite of the zero padding is exact since
    only the folded-out constant basis term is nonzero at x=0).
  - Data parallel over batch: 16 images / 8 cores.
"""

import numpy as np
import ml_dtypes

import concourse.bacc as bacc
import concourse.mybir as mybir
import concourse.tile as tile
from concourse.bass_utils import run_bass_kernel_spmd

F32 = mybir.dt.float32
BF16 = mybir.dt.bfloat16

B, CIN, H, W = 16, 64, 32, 32
COUT = 128
K2 = 9          # 3x3 taps
BASIS = 8       # Hermite orders 0..7
NFEAT = 8       # on-chip features: x^1..x^7, silu
NCORES = 8
IMGS_PER_CORE = B // NCORES  # 2
HP, WP = H + 2, W + 2        # padded 34x34
LP = HP * WP                 # 1156
L = H * W                    # 1024
NHALF = 512                  # psum free dim (half the image)

_CACHE = {}


def _hermite_coeff_matrix():
    """C[a, m] = coefficient of x^m in physicists' Hermite H_a, a,m in 0..7."""
    C = np.zeros((BASIS, BASIS), dtype=np.float64)
    C[0, 0] = 1.0
    C[1, 1] = 2.0
    for n in range(1, BASIS - 1):
        # H_{n+1} = 2 x H_n - 2 n H_{n-1}
        C[n + 1, 1:] += 2.0 * C[n, :-1]
        C[n + 1, :] -= 2.0 * n * C[n - 1, :]
    return C


def _build_program():
    """Build + compile the per-core Bass program (cached per process)."""
    if "nc" in _CACHE:
        return _CACHE["nc"]

    nc = bacc.Bacc("TRN2", target_bir_lowering=False, debug=False,
                   num_devices=NCORES)

    x_in = nc.dram_tensor("x_in", [IMGS_PER_CORE, CIN, H, W], F32,
                          kind="ExternalInput").ap()
    # weight layout: [p, (j*K2 + k)*COUT + o]; p<64 -> feature f0(j) chan p,
    # p>=64 -> feature f1(j) chan p-64
    w_in = nc.dram_tensor("w_in", [128, NPAIR * K2 * COUT], BF16,
                          kind="ExternalInput").ap()
    b_in = nc.dram_tensor("b_in", [COUT, 1], F32, kind="ExternalInput").ap()
    y_out = nc.dram_tensor("y_out", [IMGS_PER_CORE, COUT, L], F32,
                           kind="ExternalOutput").ap()

    with tile.TileContext(nc) as tc:
        _kernel_body(nc, tc, x_in, w_in, b_in, y_out)

    nc.compile()
    _CACHE["nc"] = nc
    return nc


NPAIR = 4  # feature pairs per image: (x1,x2) (x3,x4) (x5,x6) (x7,silu)


def _kernel_body(nc, tc, x_in, w_in, b_in, y_out):
    """Feature pairs packed on partitions -> all matmuls are K=128 (FWL-fast
    weight loads, full PE row utilization). Per image r, pair tile j holds
    feature f0(j) on partitions 0..63 and f1(j) on 64..127 (bf16)."""
    with (
        tc.tile_pool(name="wpool", bufs=1) as wpool,
        tc.tile_pool(name="fpool", bufs=1) as fpool,
        tc.tile_pool(name="iopool", bufs=2) as iopool,
        tc.tile_pool(name="psum", bufs=4, space="PSUM") as ppool,
    ):
        # --- weight chunk j=0 first (first matmul group needs it)
        wt = [None] * NPAIR
        w_0 = wpool.tile([128, K2 * COUT], BF16, name="w_0")
        nc.sync.dma_start(w_0, w_in[:, 0:K2 * COUT])
        wt[0] = w_0

        # --- per image: duplicated zero-padded input on both partition halves
        xpd_, d_, sig_ = [], [], []
        B = [[None] * NPAIR for _ in range(IMGS_PER_CORE)]
        for r in range(IMGS_PER_CORE):
            xpd = fpool.tile([128, LP], F32, name=f"xpd{r}")
            xp3 = xpd.rearrange("p (h w) -> p h w", w=WP)
            for half in range(2):
                nc.sync.dma_start(
                    xp3[half * CIN:(half + 1) * CIN, 1:H + 1, 1:W + 1], x_in[r])
            # zero only the padding border (keeps the big DMA off the
            # memset's critical path)
            nc.gpsimd.memset(xp3[:, 0:1, :], 0.0)
            nc.gpsimd.memset(xp3[:, H + 1:H + 2, :], 0.0)
            nc.gpsimd.memset(xp3[:, 1:H + 1, 0:1], 0.0)
            nc.gpsimd.memset(xp3[:, 1:H + 1, W + 1:W + 2], 0.0)
            xpd_.append(xpd)

        for r in range(IMGS_PER_CORE):
            d = fpool.tile([128, LP], F32, name=f"d{r}")        # [x^2; x^2]
            nc.scalar.activation(d, xpd_[r], mybir.ActivationFunctionType.Square)
            d_.append(d)
        # pair 0 for both images first, so j=0 matmuls can start ASAP
        for r in range(IMGS_PER_CORE):
            b1 = fpool.tile([128, LP], BF16, name=f"b1_{r}")    # [x; x^2]
            nc.vector.tensor_copy(b1[:CIN, :], xpd_[r][:CIN, :])
            nc.vector.tensor_copy(b1[CIN:, :], d_[r][CIN:, :])
            B[r][0] = b1

        # remaining weight chunks
        for j in range(1, NPAIR):
            w_j = wpool.tile([128, K2 * COUT], BF16, name=f"w_{j}")
            nc.sync.dma_start(w_j, w_in[:, j * K2 * COUT:(j + 1) * K2 * COUT])
            wt[j] = w_j
        bias = wpool.tile([COUT, 1], F32, name="bias")
        nc.sync.dma_start(bias, b_in)

        for r in range(IMGS_PER_CORE):
            sig = fpool.tile([128, LP], F32, name=f"sig{r}")
            nc.scalar.activation(sig[CIN:, :], xpd_[r][CIN:, :],
                                 mybir.ActivationFunctionType.Sigmoid)
            sig_.append(sig)

        p2_, p3_ = [], []
        for r in range(IMGS_PER_CORE):
            xpd, d = xpd_[r], d_[r]
            p2 = fpool.tile([128, LP], F32, name=f"p2_{r}")     # [x^3; x^4]
            nc.vector.tensor_mul(p2[:CIN, :], xpd[:CIN, :], d[:CIN, :])
            nc.vector.tensor_mul(p2[CIN:, :], d[CIN:, :], d[CIN:, :])
            b2 = fpool.tile([128, LP], BF16, name=f"b2_{r}")
            nc.vector.tensor_copy(b2, p2)
            B[r][1] = b2
            p2_.append(p2)
        for r in range(IMGS_PER_CORE):
            p3 = fpool.tile([128, LP], F32, name=f"p3_{r}")     # [x^5; x^6]
            nc.vector.tensor_mul(p3, p2_[r], d_[r])
            b3 = fpool.tile([128, LP], BF16, name=f"b3_{r}")
            nc.vector.tensor_copy(b3, p3)
            B[r][2] = b3
            p3_.append(p3)
        for r in range(IMGS_PER_CORE):
            b4 = fpool.tile([128, LP], BF16, name=f"b4_{r}")    # [x^7; silu]
            nc.vector.tensor_mul(b4[:CIN, :], p3_[r][:CIN, :], d_[r][:CIN, :])
            nc.vector.tensor_mul(b4[CIN:, :], sig_[r][CIN:, :], xpd_[r][CIN:, :])
            B[r][3] = b4

        # --- conv as accumulated K=128 matmuls
        n_acc = NPAIR * K2  # matmuls per psum tile
        for nh in range(2):  # output row halves (16 rows x 32 cols = 512)
            psums = [ppool.tile([COUT, NHALF], F32, name=f"ps{nh}_{r}", tag="ps")
                     for r in range(IMGS_PER_CORE)]
            for r in range(IMGS_PER_CORE):
                for j in range(NPAIR):
                    for k in range(K2):
                        kh, kw = divmod(k, 3)
                        cnt = j * K2 + k
                        lhsT = wt[j][:, k * COUT:(k + 1) * COUT]
                        g3 = B[r][j].rearrange("p (h w) -> p h w", w=WP)
                        rhs = g3[:, nh * 16 + kh: nh * 16 + kh + 16, kw: kw + W]
                        nc.tensor.matmul(psums[r], lhsT, rhs,
                                         start=(cnt == 0),
                                         stop=(cnt == n_acc - 1))
            for r in range(IMGS_PER_CORE):
                o_sb = iopool.tile([COUT, NHALF], F32, name=f"osb{nh}_{r}",
                                   tag="osb")
                nc.vector.tensor_scalar(o_sb, psums[r], bias, None,
                                        op0=mybir.AluOpType.add)
                nc.sync.dma_start(y_out[r, :, nh * NHALF:(nh + 1) * NHALF],
                                  o_sb)


def _prepare_host_inputs(x, w_b, w_s, c):
    """Fold Hermite->monomial transform into weights; build per-core inputs."""
    x = np.asarray(x, dtype=np.float32)
    w_b64 = np.asarray(w_b, dtype=np.float64)[..., 0]          # [i,o,k]
    w_s64 = np.asarray(w_s, dtype=np.float64)[..., 0]          # [i,o,k]
    c64 = np.asarray(c, dtype=np.float64)[:, :, :, 0, :]       # [i,o,k,a]

    cw = w_s64[..., None] * c64                                # [i,o,k,a]
    C = _hermite_coeff_matrix()                                # [a,m]
    w_mono = np.einsum("ioka,am->iokm", cw, C)                 # [i,o,k,m]

    bias = w_mono[..., 0].sum(axis=(0, 2)).astype(np.float32)  # [o]

    # w_host[p, (j*K2 + k)*COUT + o]: pair j = (f0, f1) on partition halves
    # j=0..2 -> (x^{2j+1}, x^{2j+2}); j=3 -> (x^7, silu)
    w_host = np.zeros((128, NPAIR * K2 * COUT), dtype=np.float64)
    for j in range(NPAIR):
        f0 = w_mono[:, :, :, 2 * j + 1]                        # [i,o,k]
        f1 = w_mono[:, :, :, 2 * j + 2] if j < 3 else w_b64
        blk0 = np.transpose(f0, (0, 2, 1)).reshape(CIN, K2 * COUT)
        blk1 = np.transpose(f1, (0, 2, 1)).reshape(CIN, K2 * COUT)
        w_host[:CIN, j * K2 * COUT:(j + 1) * K2 * COUT] = blk0
        w_host[CIN:, j * K2 * COUT:(j + 1) * K2 * COUT] = blk1
    w_host = w_host.astype(ml_dtypes.bfloat16)

    in_maps = []
    for core in range(NCORES):
        xs = x[core * IMGS_PER_CORE:(core + 1) * IMGS_PER_CORE]
        in_maps.append({
            "x_in": np.ascontiguousarray(xs),
            "w_in": w_host,
            "b_in": bias.reshape(COUT, 1),
        })
    return in_maps, w_host.astype(np.float64), bias


def _spot_reference(x, w_host64, bias, b_idx, n_out=16):
    """Numpy mini-reference for one image, first n_out channels (kernel math)."""
    xp = np.zeros((CIN, HP, WP), dtype=np.float64)
    xp[:, 1:H + 1, 1:W + 1] = x[b_idx].astype(np.float64)
    feats = []
    for j in range(NPAIR):
        f0 = xp ** (2 * j + 1)
        f1 = xp ** (2 * j + 2) if j < 3 else xp / (1.0 + np.exp(-xp))
        feats.append(np.concatenate([f0, f1], axis=0))   # [128, HP, WP]
    out = np.tile(bias[:n_out, None].astype(np.float64), (1, L))  # [n_out, L]
    for j in range(NPAIR):
        for k in range(K2):
            kh, kw = divmod(k, 3)
            win = feats[j][:, kh:kh + H, kw:kw + W].reshape(128, L)
            wk = w_host64[:, (j * K2 + k) * COUT:(j * K2 + k) * COUT + n_out]
            out += wk.T @ win
    return out  # [n_out, L] float64


def kernel(x, w_b, w_s, c):
    nc = _build_program()
    in_maps, w_host64, bias = _prepare_host_inputs(x, w_b, w_s, c)
    x = np.asarray(x, dtype=np.float32)

    last_err = None
    for _attempt in range(3):
        try:
            res = run_bass_kernel_spmd(nc, in_maps, core_ids=list(range(NCORES)))
        except Exception as e:  # transient tunnel/device failures
            last_err = e
            continue
        out = np.concatenate(
            [res.results[core]["y_out"].reshape(IMGS_PER_CORE, COUT, H, W)
             for core in range(NCORES)], axis=0).astype(np.float32)
        # guard against transient device garbage: spot-check 1 image per core
        ok = np.isfinite(out).all()
        if ok:
            for core in range(NCORES):
                b_idx = core * IMGS_PER_CORE
                ref = _spot_reference(x, w_host64, bias, b_idx)
                got = out[b_idx, :16].reshape(16, L).astype(np.float64)
                err = np.linalg.norm(got - ref) / (np.linalg.norm(ref) + 1e-30)
                if not np.isfinite(err) or err > 3e-2:
                    ok = False
                    break
        if ok:
            return out
    raise RuntimeError(
        f"kernel: device output failed spot-check after 3 attempts ({last_err})")
